# revision 6
# baseline (speedup 1.0000x reference)
"""Trainium2 Bass kernel v3.6 for nn_Decoder (2 queues + sliced fast-start).

v3.1 (kernel3b) plus: (a) fast-start mini-decode of group 0's first 8 codes so
the serial Pool gather stream starts ~4us earlier; (b) assembly and stores
split by (p, k-half) so the last group's store pipeline drains sooner.
"""

import numpy as np

import concourse.bacc as bacc
import concourse.bass as bass
import concourse.mybir as mybir
import concourse.tile as tile

BATCH = 8192
XCOLS = 512
NCODE = 23
NBITS = 22
L = 131072
ROW = 768
NCORES = 8
BC = BATCH // NCORES
P = 128
GROUPS = BC // P
OD = 2 * 48 * 128
NNAR = 14
NWID = 9
NFAST = 8            # codes decoded by the fast-start mini-chain (group 0)

f16 = mybir.dt.float16
f32 = mybir.dt.float32
i32 = mybir.dt.int32


def _decode(nc, xpool, spool, xf, w_tile, cods, tag):
    """Decode codes `cods` (slice) from xf -> presigned row offsets [P, len].

    `tag` must be one of a FIXED small set ("f", "r", "g") — tile pools
    allocate a separate `bufs`-deep ring per distinct tag.
    """
    lo, hi = cods
    n = hi - lo
    prod = xpool.tile([P, n * NBITS], f32, tag=f"prod{tag}")
    nc.vector.tensor_tensor(
        out=prod[:], in0=xf[:, 6 + lo * NBITS : 6 + hi * NBITS],
        in1=w_tile[:, lo * NBITS : hi * NBITS],
        op=mybir.AluOpType.mult,
    )
    codes = spool.tile([P, n], f32, tag=f"codes{tag}")
    nc.vector.tensor_reduce(
        out=codes[:], in_=prod[:].rearrange("n (c a) -> n c a", a=NBITS),
        axis=mybir.AxisListType.X, op=mybir.AluOpType.add,
    )
    codesi = spool.tile([P, n], i32, tag=f"codesi{tag}")
    nc.vector.tensor_copy(out=codesi[:], in_=codes[:])
    tti = spool.tile([P, n], i32, tag=f"tti{tag}")
    nc.vector.tensor_scalar(
        out=tti[:], in0=codesi[:], scalar1=L, scalar2=L,
        op0=mybir.AluOpType.is_gt, op1=mybir.AluOpType.mult,
    )
    idx = spool.tile([P, n], i32, tag=f"idx{tag}")
    nc.vector.tensor_scalar(
        out=idx[:], in0=codesi[:], scalar1=L - 1, scalar2=None,
        op0=mybir.AluOpType.bitwise_and,
    )
    idxp = spool.tile([P, n], i32, tag=f"idxp{tag}")
    nc.vector.tensor_tensor(
        out=idxp[:], in0=idx[:], in1=tti[:], op=mybir.AluOpType.add,
    )
    return idxp


def build_module():
    nc = bacc.Bacc(
        "TRN2", target_bir_lowering=False, debug=False, num_swdge_queues=2,
    )
    x_t = nc.dram_tensor("x", [BC, XCOLS], i32, kind="ExternalInput")
    tab_t = nc.dram_tensor("tabs", [2 * L, ROW], f16, kind="ExternalInput")
    w_t = nc.dram_tensor("w", [P, NCODE * NBITS], f32, kind="ExternalInput")
    out_t = nc.dram_tensor("out", [BC, OD], f16, kind="ExternalOutput")

    with tile.TileContext(nc) as tc:
        with (
            tc.tile_pool(name="const", bufs=1) as cpool,
            tc.tile_pool(name="xp", bufs=2) as xpool,
            tc.tile_pool(name="sm", bufs=GROUPS) as spool,
            tc.tile_pool(name="gn", bufs=3) as npool,
            tc.tile_pool(name="gw", bufs=4) as wpool,
            tc.tile_pool(name="op", bufs=2) as opool,
        ):
            w_tile = cpool.tile([P, NCODE * NBITS], f32)
            nc.sync.dma_start(w_tile[:], w_t[:])

            # Phase 1: decode. Group 0's first NFAST codes go through a short
            # chain so the Pool stream starts as early as possible.
            idxs = []       # per group: list of (idxp_tile, code_lo)
            for g in range(GROUPS):
                b0 = g * P
                x_tile = xpool.tile([P, XCOLS], i32)
                if g == 0:
                    nfc = 6 + NFAST * NBITS
                    nc.scalar.dma_start(x_tile[:, 0:nfc], x_t[b0 : b0 + P, 0:nfc])
                    nc.scalar.dma_start(
                        x_tile[:, nfc:XCOLS], x_t[b0 : b0 + P, nfc:XCOLS]
                    )
                else:
                    nc.scalar.dma_start(x_tile[:], x_t[b0 : b0 + P, :])
                xf = xpool.tile([P, XCOLS], f32)
                if g == 0:
                    nfc = 6 + NFAST * NBITS
                    nc.vector.tensor_copy(out=xf[:, 0:nfc], in_=x_tile[:, 0:nfc])
                    fast = _decode(nc, xpool, spool, xf, w_tile, (0, NFAST), "f")
                    nc.vector.tensor_copy(out=xf[:, nfc:XCOLS], in_=x_tile[:, nfc:XCOLS])
                    rest = _decode(nc, xpool, spool, xf, w_tile, (NFAST, NCODE), "r")
                    idxs.append([(fast, 0), (rest, NFAST)])
                else:
                    nc.vector.tensor_copy(out=xf[:], in_=x_tile[:])
                    full = _decode(nc, xpool, spool, xf, w_tile, (0, NCODE), "g")
                    idxs.append([(full, 0)])

            # Phase 2: free-running gather stream + assembly + stores.
            for g in range(GROUPS):
                b0 = g * P
                gcn = npool.tile([P, NNAR * ROW], f16)
                gcw = wpool.tile([P, NWID * ROW], f16)
                for c in range(NCODE):
                    for idxp, lo in idxs[g]:
                        if lo <= c < lo + idxp.shape[1]:
                            off = idxp[:, c - lo : c - lo + 1]
                            break
                    dst = (
                        gcn[:, c * ROW : (c + 1) * ROW]
                        if c < NNAR
                        else gcw[:, (c - NNAR) * ROW : (c - NNAR + 1) * ROW]
                    )
                    gi = nc.gpsimd.indirect_dma_start(
                        out=dst, out_offset=None, in_=tab_t[:],
                        in_offset=bass.IndirectOffsetOnAxis(ap=off, axis=0),
                    )
                    if c % 2:
                        gi.ins.queue = "qPoolDynamic1"
                gvn = gcn[:].rearrange("n (c p k h) -> n c p k h", c=NNAR, p=2, k=48)
                gvw = gcw[:].rearrange("n (c p k h) -> n c p k h", c=NWID, p=2, k=48)
                od = opool.tile([P, OD], f16)
                od5 = od[:].rearrange("n (p k s h) -> n p k s h", p=2, k=48, s=16)
                q = 0
                for p in range(2):
                    for k0 in (0, 24):
                        ks = slice(k0, k0 + 24)
                        nc.vector.tensor_copy(
                            out=od5[:, p, ks, 0:7, 0:4],
                            in_=gvn[:, 0:7, p, ks, 0:4].rearrange(
                                "n c k h -> n k c h"),
                        )
                        nc.scalar.activation(
                            out=od5[:, p, ks, 0:7, 4:8],
                            in_=gvn[:, 7:14, p, ks, 4:8].rearrange(
                                "n c k h -> n k c h"),
                            func=mybir.ActivationFunctionType.Identity,
                        )
                        nc.vector.tensor_copy(
                            out=od5[:, p, ks, 7:16, :],
                            in_=gvw[:, :, p, ks].rearrange("n c k h -> n k c h"),
                        )
                        # small store chunks: keep SDMA packets short so
                        # in-flight gather descriptors get serviced sooner
                        # (gather completion latency gates the Pool stream)
                        o0 = (p * 2 + (k0 // 24)) * (OD // 4)
                        half = OD // 8
                        for h in range(2):
                            eng = nc.sync if q % 2 else nc.scalar
                            s0 = o0 + h * half
                            eng.dma_start(
                                out=out_t[b0 : b0 + P, s0 : s0 + half],
                                in_=od[:, s0 : s0 + half],
                            )
                            q += 1
    nc.compile()
    return nc


def make_weights():
    w = np.tile((2.0 ** np.arange(NBITS)).astype(np.float32), NCODE)
    return np.broadcast_to(w, (P, NCODE * NBITS)).copy()


def make_tabs(table):
    t = np.asarray(table).reshape(L, ROW)
    return np.concatenate([t, np.float16(1.0) - t], axis=0)


def make_in_maps(x, table):
    tabs = make_tabs(table)
    w = make_weights()
    return [
        {
            "x": np.ascontiguousarray(x[i * BC : (i + 1) * BC]),
            "tabs": tabs,
            "w": w,
        }
        for i in range(NCORES)
    ]


_NC_CACHE = None


def _get_module():
    global _NC_CACHE
    if _NC_CACHE is None:
        _NC_CACHE = build_module()
    return _NC_CACHE


def assemble_output(core_outs):
    data = np.concatenate(core_outs, axis=0).reshape(BATCH, 2, 48, 128)
    out = np.full((BATCH, 2, 126, 128), 0.5, dtype=np.float32)
    out[:, :, 19:67, :] = data
    return out


def kernel(x: np.ndarray, table: np.ndarray) -> np.ndarray:
    from concourse.bass_utils import run_bass_kernel_spmd

    x = np.asarray(x)
    table = np.asarray(table)
    assert x.shape == (BATCH, XCOLS) and table.shape == (L, 2, 48, 8)
    nc = _get_module()
    res = run_bass_kernel_spmd(nc, make_in_maps(x, table), core_ids=list(range(NCORES)))
    return assemble_output([res.results[i]["out"] for i in range(NCORES)])
